# revision 1
# baseline (speedup 1.0000x reference)
"""Causal multi-head self-attention kernel for Trainium2 (8 NeuronCores).

Problem: x [8, 32, 512, 64], 4 heads x head_dim 16, causal softmax attention
per (batch, chunk), projections Wq/Wk/Wv/Wo [64, 64] applied as x @ W.T.

Sharding: data-parallel over batch (8 batch elements -> 8 cores, 32 chunks
each). Weights replicated.

Device layout strategy (per chunk of [512, 64]):
  - Host pre-transposes x to xT [64, 512] so projections run directly on PE.
  - qT/kT computed in a head-PADDED layout [128, 512]: row 32h+d holds head h,
    dim d (d<16). This makes per-head [16, *] slices land on 32-row PE bands so
    4 heads' score matmuls run concurrently via tile_position row tiling.
  - scoresT[k, q] per head computed block-wise over k (4 key-blocks of 128),
    with the q range trimmed causally to q >= k_block_start.
  - exp via ScalarE activation (fused 1/sqrt(head_dim) scale), one instruction
    per key-block covering all 4 heads (psum banks are contiguous).
  - The diagonal 128x128 block of each (head, key-block) is masked AFTER exp
    by multiplying with a host-provided upper-triangular 0/1 mask.
  - attended.T computed with col-tiled matmuls: stationary = v packed
    [keys, 32] per head with a ones column (yields softmax denominators for
    free) and zero columns (keeps psum rows clean); accumulated over
    key-blocks into one psum bank [128, 512] (rows 32h+d = attT_h, row
    32h+16 = sum of exp).
  - normalize: reciprocal of sums (fast DVE approx), DMA-broadcast across
    rows, elementwise multiply.
  - output projection with zero-padded WoT rows kills the helper rows.
"""

import sys

if "/opt/trn_rl_repo" not in sys.path:
    sys.path.insert(0, "/opt/trn_rl_repo")

import numpy as np

import concourse.bass as bass
import concourse.mybir as mybir
import concourse.tile as tile
from concourse import bacc
from concourse.bass_utils import run_bass_kernel_spmd

NUM_HEADS = 4
HEAD_DIM = 16
D = 64
SEQ = 512
N_CORES = 8
F32 = mybir.dt.float32
BF16 = mybir.dt.float16
F32R = mybir.dt.float32r


def _r(ap):
    return ap.bitcast(F32R)


def build_nc(n_chunks: int, n_repeat: int = 1, hw_loop: int = 1):
    nc = bacc.Bacc("TRN2", target_bir_lowering=False, debug=False)
    xt_d = nc.declare_dram_parameter("xt", [n_chunks, D, SEQ], F32R, isOutput=False)
    wqt_d = nc.declare_dram_parameter("wqt", [D, 128], F32R, isOutput=False)
    wkt_d = nc.declare_dram_parameter("wkt", [D, 128], F32R, isOutput=False)
    wvt_d = nc.declare_dram_parameter("wvt", [D, D], F32R, isOutput=False)
    wot_d = nc.declare_dram_parameter("wot", [128, D], F32R, isOutput=False)
    mask_d = nc.declare_dram_parameter("mask", [128, 128], F32, isOutput=False)
    y_d = nc.declare_dram_parameter("y", [n_chunks, SEQ, D], F32, isOutput=True)

    with tile.TileContext(nc) as tc:
        with (
            tc.tile_pool(name="consts", bufs=1) as consts,
            tc.tile_pool(name="xin", bufs=3) as xin,
            tc.tile_pool(name="qkts", bufs=2) as qkts_pool,
            tc.tile_pool(name="vpkp", bufs=12) as vpkp,
            tc.tile_pool(name="expp", bufs=4) as expp,
            tc.tile_pool(name="attp", bufs=2) as attp,
            tc.tile_pool(name="outp", bufs=2) as outp,
            tc.tile_pool(name="dramp", bufs=2, space="DRAM") as dramp,
            tc.tile_pool(name="ps_misc", bufs=2, space="PSUM") as ps_misc,
            tc.tile_pool(name="ps_sc", bufs=2, space="PSUM") as ps_sc,
            tc.tile_pool(name="ps_att", bufs=2, space="PSUM") as ps_att,
        ):
            wqt = consts.tile([D, 128], F32R)
            nc.sync.dma_start(out=wqt[:, :], in_=wqt_d[:, :])
            wkt = consts.tile([D, 128], F32R)
            nc.sync.dma_start(out=wkt[:, :], in_=wkt_d[:, :])
            wvt = consts.tile([D, D], F32R)
            nc.sync.dma_start(out=wvt[:, :], in_=wvt_d[:, :])
            wot = consts.tile([128, D], F32R)
            nc.sync.dma_start(out=wot[:, :], in_=wot_d[:, :])
            mask_f = consts.tile([128, 128], F32)
            nc.sync.dma_start(out=mask_f[:, :], in_=mask_d[:, :])
            mask = consts.tile([128, 128], BF16)
            nc.vector.tensor_copy(mask[:, :], mask_f[:, :])
            expbias = consts.tile([128, 1], F32)
            nc.vector.memset(expbias[:, :], -8.0)

            # persistent rotating vpk tiles: the ones/zero helper columns are
            # written once; only the v columns are refreshed per key-block
            N_VPK = 12
            vpk_tiles = []
            for i in range(N_VPK):
                vt = vpkp.tile([128, 128], BF16, tag="vpk")
                v3 = vt[:, :].rearrange("p (h c) -> p h c", h=4)
                nc.vector.memset(v3[:, :, HEAD_DIM : HEAD_DIM + 1], 1.0)
                nc.vector.memset(v3[:, :, HEAD_DIM + 1 : 32], 0.0)
                vpk_tiles.append(vt)

            # ---------------------------------------------------------
            # Software pipeline (per emission iteration c):
            #   1. late-tail(c-2): attn-mul, output projection, store
            #   2. head(c+1): qkv projections + sbuf copies (hoisted a full
            #      chunk early so scores never wait on them)
            #   3. attention(c): per-wave scores->exp->mask, with the
            #      attended matmuls delayed one wave so the in-order PE
            #      stream never blocks on exp
            #   4. early-tail(c-1): softmax-sum reciprocal + DRAM-bounce
            #      broadcast (runs while chunk c computes)
            # ---------------------------------------------------------
            xt_tiles = {}

            def load_xt(ci):
                t = xin.tile([D, SEQ], F32R, tag="xt")
                nc.sync.dma_start(out=t[:, :], in_=xt_d[ci % n_chunks])
                xt_tiles[ci] = t

            heads = {}       # c -> (qts, kts, vpks)
            attps = {}       # c -> att_ps psum tile
            pend_att = []  # delayed attended waves: [(c, kb, pr)]
            rbs = {}         # c -> (att_ps, rb)

            def emit_attended(c, kb, pr):
                q0 = kb * 128
                n_q = SEQ - q0
                _, _, vpks = heads[c]
                exps = exp_tiles[(c, kb, pr)]
                for m in range(2):
                    h = 2 * pr + m
                    nc.tensor.matmul(
                        attps[c][32 * h : 32 * h + 32, q0:SEQ],
                        vpks[kb][:, 32 * h : 32 * h + 32],
                        exps[:, m * n_q : (m + 1) * n_q],
                        start=(kb == 0),
                        stop=(kb == 3),
                        tile_position=(0, 32 * h),
                    )
                del exp_tiles[(c, kb, pr)]

            exp_tiles = {}

            def emit_pipeline():
                n_virt = n_chunks * n_repeat
                load_xt(0)
                load_xt(1)
                for c in range(n_virt + 2):
                    # ---- 1. late tail for chunk c-2
                    if 0 <= c - 2:
                        p_att, p_rb = rbs.pop(c - 2)
                        attn = attp.tile([128, SEQ], F32R, tag="attn")
                        nc.vector.tensor_mul(attn[:, :], p_att[:, :], p_rb[:, :])
                        o_ps = ps_misc.tile([128, 4 * D], F32, tag="mm")
                        for sb in range(4):
                            nc.tensor.matmul(
                                o_ps[:, sb * D : (sb + 1) * D],
                                attn[:, sb * 128 : (sb + 1) * 128],
                                wot[:, :],
                                start=True,
                                stop=True,
                            )
                        outs = outp.tile([128, 4 * D], F32, tag="outs")
                        nc.vector.tensor_copy(outs[:, :], o_ps[:, :])
                        nc.sync.dma_start(
                            out=y_d[(c - 2) % n_chunks].rearrange("(sb p) o -> p sb o", sb=4),
                            in_=outs[:, :].rearrange("p (sb o) -> p sb o", sb=4),
                        )

                    # ---- 2. head for chunk c+1
                    if c + 1 < n_virt:
                        hc = c + 1
                        xt = xt_tiles.pop(hc)
                        if hc + 1 < n_virt:
                            load_xt(hc + 1)
                        qt_ps = ps_misc.tile([128, SEQ], F32, tag="mm")
                        nc.tensor.matmul(
                            qt_ps[:, :], wqt[:, :], xt[:, :],
                            start=True, stop=True,
                        )
                        kt_ps = ps_misc.tile([128, SEQ], F32, tag="mm")
                        nc.tensor.matmul(
                            kt_ps[:, :], wkt[:, :], xt[:, :],
                            start=True, stop=True,
                        )
                        qts = qkts_pool.tile([128, SEQ], F32R, tag="qts")
                        nc.vector.tensor_copy(qts[:, :], qt_ps[:, :])
                        kts = qkts_pool.tile([128, SEQ], F32R, tag="kts")
                        nc.vector.tensor_copy(kts[:, :], kt_ps[:, :])
                        v_ps = ps_misc.tile([128, 4 * D], F32, tag="mm")
                        for sb in range(4):
                            nc.tensor.matmul(
                                v_ps[:, sb * D : (sb + 1) * D],
                                xt[:, sb * 128 : (sb + 1) * 128],
                                wvt[:, :],
                                start=True,
                                stop=True,
                            )
                        vpks = []
                        for kb in range(4):
                            vpk = vpk_tiles[(hc % 3) * 4 + kb]
                            v3o = vpk[:, :].rearrange("p (h c) -> p h c", h=4)
                            v3i = v_ps[:, kb * D : (kb + 1) * D].rearrange(
                                "p (h d) -> p h d", h=4
                            )
                            nc.vector.tensor_copy(v3o[:, :, 0:HEAD_DIM], v3i)
                            vpks.append(vpk)
                        heads[hc] = (qts, kts, vpks)
                    elif c == 0 and n_virt == 1:
                        pass

                    # bootstrap: chunk 0's head is emitted at c == 0 too
                    if c == 0 and n_virt > 0:
                        hc = 0
                        xt = xt_tiles.pop(0)
                        qt_ps = ps_misc.tile([128, SEQ], F32, tag="mm")
                        nc.tensor.matmul(
                            qt_ps[:, :], wqt[:, :], xt[:, :],
                            start=True, stop=True,
                        )
                        kt_ps = ps_misc.tile([128, SEQ], F32, tag="mm")
                        nc.tensor.matmul(
                            kt_ps[:, :], wkt[:, :], xt[:, :],
                            start=True, stop=True,
                        )
                        qts = qkts_pool.tile([128, SEQ], F32R, tag="qts")
                        nc.vector.tensor_copy(qts[:, :], qt_ps[:, :])
                        kts = qkts_pool.tile([128, SEQ], F32R, tag="kts")
                        nc.vector.tensor_copy(kts[:, :], kt_ps[:, :])
                        v_ps = ps_misc.tile([128, 4 * D], F32, tag="mm")
                        for sb in range(4):
                            nc.tensor.matmul(
                                v_ps[:, sb * D : (sb + 1) * D],
                                xt[:, sb * 128 : (sb + 1) * 128],
                                wvt[:, :],
                                start=True,
                                stop=True,
                            )
                        vpks = []
                        for kb in range(4):
                            vpk = vpk_tiles[0 * 4 + kb]
                            v3o = vpk[:, :].rearrange("p (h c) -> p h c", h=4)
                            v3i = v_ps[:, kb * D : (kb + 1) * D].rearrange(
                                "p (h d) -> p h d", h=4
                            )
                            nc.vector.tensor_copy(v3o[:, :, 0:HEAD_DIM], v3i)
                            vpks.append(vpk)
                        heads[0] = (qts, kts, vpks)

                    # ---- 3. attention waves for chunk c
                    if c < n_virt:
                        qts, kts, vpks = heads[c]
                        att_ps = ps_att.tile([128, SEQ], F32, tag="att")
                        attps[c] = att_ps
                        wave = 0
                        for kb in range(4):
                            q0 = kb * 128
                            n_q = SEQ - q0
                            for pr in range(2):
                                sc_ps = ps_sc.tile([128, 2 * SEQ], F32, tag="sc")
                                for m in range(2):
                                    h = 2 * pr + m
                                    nc.tensor.matmul(
                                        sc_ps[:, m * SEQ : m * SEQ + n_q],
                                        kts[32 * h : 32 * h + HEAD_DIM,
                                            q0 : q0 + 128],
                                        qts[32 * h : 32 * h + HEAD_DIM, q0:SEQ],
                                        start=True,
                                        stop=True,
                                        tile_position=(32 * h, 0),
                                    )
                                exps = expp.tile([128, 2 * SEQ], BF16, tag="exps")
                                exp_tiles[(c, kb, pr)] = exps
                                e3 = exps[:, 0 : 2 * n_q].rearrange(
                                    "p (h n) -> p h n", h=2
                                )
                                s3 = sc_ps[:, :].rearrange("p (h n) -> p h n", h=2)[
                                    :, :, 0:n_q
                                ]
                                # constant shift keeps exp in fp16 range
                                # (softmax is shift-invariant; the factor e^-8
                                # cancels between numerator and denominator)
                                nc.scalar.activation(
                                    e3,
                                    s3,
                                    mybir.ActivationFunctionType.Exp,
                                    scale=1.0 / np.sqrt(HEAD_DIM),
                                    bias=expbias[:, :],
                                )
                                diag = e3[:, :, 0:128]
                                m_ap = mask[:, :]
                                m_bc = bass.AP(
                                    tensor=m_ap.tensor,
                                    offset=m_ap.offset,
                                    ap=[m_ap.ap[0], [0, 2], m_ap.ap[1]],
                                )
                                meng = nc.vector if wave % 2 == 0 else nc.gpsimd
                                wave += 1
                                meng.tensor_mul(diag, diag, m_bc)
                                # delayed attended: keep 2 waves in flight so
                                # the mask latency never bubbles the PE stream
                                pend_att.append((c, kb, pr))
                                if len(pend_att) > 2:
                                    emit_attended(*pend_att.pop(0))
                    else:
                        while pend_att:
                            emit_attended(*pend_att.pop(0))

                    # ---- 4. early tail for chunk c-1 (after its attended flush)
                    tc_ = c - 1
                    if 0 <= tc_ < n_virt:
                        while pend_att and pend_att[0][0] == tc_:
                            emit_attended(*pend_att.pop(0))
                        p_att = attps.pop(tc_)
                        recs = attp.tile([128, SEQ], F32, tag="recs")
                        nc.vector.reciprocal_approx_fast(
                            out=recs[:, :], in_=p_att[:, :]
                        )
                        scr = dramp.tile([4, SEQ], F32, tag="scr")
                        for h in range(4):
                            nc.sync.dma_start(
                                out=scr[h : h + 1, :],
                                in_=recs[32 * h + HEAD_DIM : 32 * h + HEAD_DIM + 1, :],
                            )
                        rb = attp.tile([128, SEQ], F32, tag="rb")
                        scr_ap = scr[:, :]
                        scr_bc = bass.AP(
                            tensor=scr_ap.tensor,
                            offset=scr_ap.offset,
                            ap=[[SEQ, 4], [0, 32], [1, SEQ]],
                        )
                        nc.gpsimd.dma_start(out=rb[:, :], in_=scr_bc)
                        rbs[tc_] = (p_att, rb)

            if hw_loop > 1:
                with tc.For_i(0, hw_loop, 1):
                    emit_pipeline()
            else:
                emit_pipeline()
    nc.compile()
    return nc


def _prep_weights(Wq, Wk, Wv, Wo):
    # wqt[i, 32h+d] = Wq[16h+d, i] (zero-padded rows 16..31 of each band)
    def pad_wt(W):
        wt = np.zeros((D, 128), dtype=np.float32)
        wt.reshape(D, 4, 32)[:, :, :HEAD_DIM] = W.T.reshape(D, 4, HEAD_DIM)
        return wt

    wqt = pad_wt(np.asarray(Wq, dtype=np.float32))
    wkt = pad_wt(np.asarray(Wk, dtype=np.float32))
    wvt = np.ascontiguousarray(np.asarray(Wv, dtype=np.float32).T)
    # wot[32h+d, o] = Wo[o, 16h+d]; helper rows (d>=16) zero
    wot = np.zeros((128, D), dtype=np.float32)
    wot.reshape(4, 32, D)[:, :HEAD_DIM, :] = (
        np.asarray(Wo, dtype=np.float32).T.reshape(4, HEAD_DIM, D)
    )
    mask = np.triu(np.ones((128, 128), dtype=np.float32))
    return wqt, wkt, wvt, wot, mask


_NC_CACHE = {}


def _get_nc(n_chunks, n_repeat=1, hw_loop=1):
    key = (n_chunks, n_repeat, hw_loop)
    if key not in _NC_CACHE:
        _NC_CACHE[key] = build_nc(n_chunks, n_repeat, hw_loop)
    return _NC_CACHE[key]


def run(x, Wq, Wk, Wv, Wo, trace=False, n_repeat=1):
    x = np.asarray(x, dtype=np.float32)
    B, C, S, d = x.shape
    assert (B, S, d) == (N_CORES, SEQ, D), f"unexpected shape {x.shape}"
    wqt, wkt, wvt, wot, mask = _prep_weights(Wq, Wk, Wv, Wo)
    nc = _get_nc(C, n_repeat)
    # xT per core: [C, 64, 512]
    in_maps = []
    for b in range(N_CORES):
        xt = np.ascontiguousarray(x[b].transpose(0, 2, 1))
        in_maps.append(
            {"xt": xt, "wqt": wqt, "wkt": wkt, "wvt": wvt, "wot": wot, "mask": mask}
        )
    res = run_bass_kernel_spmd(nc, in_maps, list(range(N_CORES)), trace=trace)
    y = np.stack([res.results[b]["y"] for b in range(N_CORES)], axis=0)
    return y, res


def kernel(x, Wq, Wk, Wv, Wo):
    y, _ = run(x, Wq, Wk, Wv, Wo, trace=False)
    return y



# revision 5
# speedup vs baseline: 9.3114x; 9.3114x over previous
"""Causal multi-head self-attention kernel for Trainium2 (8 NeuronCores), v2.

Problem: x [8, 32, 512, 64], 4 heads x head_dim 16, causal softmax attention
per (batch, chunk), projections Wq/Wk/Wv/Wo [64, 64] applied as x @ W.T.

Sharding: data-parallel over batch (8 batch elements -> 8 cores, 32 chunks
each). Weights replicated.

v2 design (per chunk of [512, 64]):
  - All PE operands fp16 (1 cycle/col at any free size; fp32r pays 4x below
    256 cols). Host pre-converts x/weights to fp16 and transposes x.
  - qT/kT in head-padded [128, 512] layout (row 32h+d = head h dim d, d<16);
    per-head score matmuls run on 32-row PE bands via tile_position.
  - 6 score waves per chunk: (kb0,pr0)(kb0,pr1)(kb1,pr0)(kb1,pr1) as 2-head
    waves, kb2/kb3 as merged 4-head waves -> fewer, larger ScalarE exp calls
    (exp is the co-bottleneck: 1 elem/cycle/lane @1.2GHz + ~370ns/call).
  - exp via ScalarE (scale 1/4, bias -8; softmax shift-invariant) -> fp16.
  - causal mask on the 128x128 diag blocks AFTER exp: DVE multiply in fp16
    (2x perf mode), one op per wave covering all heads of the wave.
  - attended via col-tiled matmuls: stationary vpk [128, 32] per (kb, head)
    = [v_h | ones | zeros]; the ones column yields softmax denominators in
    att_ps row 32h+16 for free. Accumulated over kb into att_ps [128, 512].
  - normalize: reciprocal of the 4 denominator rows (strided-partition AP),
    one SBUF->SBUF broadcast DMA to all 128 rows, one DVE multiply
    -> attn fp16 (also the PSUM->SBUF move for the output projection).
  - output projection TRANSPOSED: yT [64, 512] = wot.T @ attn in ONE matmul
    (moving 512 fp16 cols); DMA to DRAM contiguous; host transposes back.
  - PSUM budget exactly 8 banks: qkv staging 1 (q -> copy -> k -> copy -> v
    serialized through one bank), scores 2x[128,1024] double-buffered 4,
    att 2x[128,512] double-buffered 2, yT 1.
"""

import sys

if "/opt/trn_rl_repo" not in sys.path:
    sys.path.insert(0, "/opt/trn_rl_repo")

import numpy as np

import concourse.bass as bass
import concourse.mybir as mybir
import concourse.tile as tile
from concourse import bacc
from concourse.bass_utils import run_bass_kernel_spmd

NUM_HEADS = 4
HEAD_DIM = 16
D = 64
SEQ = 512
N_CORES = 8
F32 = mybir.dt.float32
F16 = mybir.dt.float16

# wave list: (kb, heads) — heads listed explicitly; n_q = SEQ - 128*kb
WAVES = [
    (0, (0, 1)),
    (0, (2, 3)),
    (1, (0, 1)),
    (1, (2, 3)),
    (2, (0, 1, 2, 3)),
    (3, (0, 1, 2, 3)),
]


def build_nc(n_chunks: int, n_repeat: int = 1, hw_loop: int = 1):
    nc = bacc.Bacc("TRN2", target_bir_lowering=False, debug=False)
    xt_d = nc.declare_dram_parameter("xt", [n_chunks, D, SEQ], F16, isOutput=False)
    wqt_d = nc.declare_dram_parameter("wqt", [D, 128], F16, isOutput=False)
    wkt_d = nc.declare_dram_parameter("wkt", [D, 128], F16, isOutput=False)
    wvt_d = nc.declare_dram_parameter("wvt", [D, D], F16, isOutput=False)
    wot_d = nc.declare_dram_parameter("wot", [128, D], F16, isOutput=False)
    mask_d = nc.declare_dram_parameter("mask", [128, 128], F16, isOutput=False)
    # yT layout [64, 512] per chunk; host transposes back
    y_d = nc.declare_dram_parameter("y", [n_chunks, D, SEQ], F32, isOutput=True)

    with tile.TileContext(nc) as tc:
        with (
            tc.tile_pool(name="consts", bufs=1) as consts,
            tc.tile_pool(name="xin", bufs=3) as xin,
            tc.tile_pool(name="qks", bufs=2) as qks_pool,
            tc.tile_pool(name="vpkp", bufs=3) as vpkp,
            tc.tile_pool(name="expp", bufs=4) as expp,
            tc.tile_pool(name="attnp", bufs=2) as attnp,
            tc.tile_pool(name="recp", bufs=2) as recp,
            tc.tile_pool(name="ps", bufs=1, space="PSUM") as ps,
        ):
            wqt = consts.tile([D, 128], F16)
            nc.sync.dma_start(out=wqt[:, :], in_=wqt_d[:, :])
            wkt = consts.tile([D, 128], F16)
            nc.sync.dma_start(out=wkt[:, :], in_=wkt_d[:, :])
            wvt = consts.tile([D, D], F16)
            nc.sync.dma_start(out=wvt[:, :], in_=wvt_d[:, :])
            wot = consts.tile([128, D], F16)
            nc.sync.dma_start(out=wot[:, :], in_=wot_d[:, :])
            mask = consts.tile([128, 128], F16)
            nc.sync.dma_start(out=mask[:, :], in_=mask_d[:, :])
            expbias = consts.tile([128, 1], F32)
            nc.vector.memset(expbias[:, :], -8.0)

            # persistent PSUM tiles (8 banks exactly)
            qkv_ps = ps.tile([128, SEQ], F32, tag="qkv", name="qkv_ps")  # 1 bank
            sc_ps = [
                ps.tile([128, 1024], F32, tag="scA", name="scA_ps"),  # 2 banks
                ps.tile([128, 1024], F32, tag="scB", name="scB_ps"),  # 2 banks
            ]
            att_ps = [
                ps.tile([128, SEQ], F32, tag="attA", name="attA_ps"),  # 1 bank
                ps.tile([128, SEQ], F32, tag="attB", name="attB_ps"),  # 1 bank
            ]
            yt_ps = ps.tile([D, SEQ], F32, tag="yt", name="yt_ps")  # 1 bank

            # persistent vpk tiles: [128, kb, h, 32] = [v_h | ones | zeros]
            N_VPK = 3
            vpk_tiles = []
            for i in range(N_VPK):
                vt = vpkp.tile([128, 4, 4, 32], F16, tag="vpk")
                nc.vector.memset(vt[:, :, :, HEAD_DIM : HEAD_DIM + 1], 1.0)
                nc.vector.memset(vt[:, :, :, HEAD_DIM + 1 : 32], 0.0)
                vpk_tiles.append(vt)

            xt_tiles = {}

            def load_xt(ci):
                t = xin.tile([D, SEQ], F16, tag="xt")
                nc.sync.dma_start(out=t[:, :], in_=xt_d[ci % n_chunks])
                xt_tiles[ci] = t

            heads = {}     # c -> (qts, kts, vpk)
            exp_tiles = {}
            pend_att = []

            def emit_attended(c, wi):
                kb, hs = WAVES[wi]
                q0 = kb * 128
                n_q = SEQ - q0
                vpk = heads[c][2]
                exps = exp_tiles.pop((c, wi))
                ap = att_ps[c % 2]
                for j, h in enumerate(hs):
                    nc.tensor.matmul(
                        ap[32 * h : 32 * h + 32, q0:SEQ],
                        vpk[:, kb, h, :],
                        exps[:, j * n_q : (j + 1) * n_q],
                        start=(kb == 0),
                        stop=(kb == 3),
                        tile_position=(0, 32 * h),
                    )

            def emit_head_q(hc):
                xt = xt_tiles[hc]
                nc.tensor.matmul(
                    qkv_ps[:, :], wqt[:, :], xt[:, :], start=True, stop=True
                )
                qts = qks_pool.tile([128, SEQ], F16, tag="qts")
                nc.vector.tensor_copy(qts[:, :], qkv_ps[:, :])
                return qts

            def emit_head_k(hc):
                xt = xt_tiles[hc]
                nc.tensor.matmul(
                    qkv_ps[:, :], wkt[:, :], xt[:, :], start=True, stop=True
                )
                kts = qks_pool.tile([128, SEQ], F16, tag="kts")
                nc.vector.tensor_copy(kts[:, :], qkv_ps[:, :])
                return kts

            def emit_head_v(hc):
                xt = xt_tiles.pop(hc)
                for sb in range(4):
                    nc.tensor.matmul(
                        qkv_ps[:, sb * D : (sb + 1) * D],
                        xt[:, sb * 128 : (sb + 1) * 128],
                        wvt[:, :],
                        start=True,
                        stop=True,
                    )
                vpk = vpk_tiles[hc % N_VPK]
                v4 = qkv_ps[:, 0 : 4 * D].rearrange("p (kb h d) -> p kb h d", kb=4, h=4)
                nc.vector.tensor_copy(vpk[:, :, :, 0:HEAD_DIM], v4)
                return vpk

            def emit_wave(c, wi):
                kb, hs = WAVES[wi]
                q0 = kb * 128
                n_q = SEQ - q0
                qts, kts, _ = heads[c]
                sc = sc_ps[wi % 2]
                # per-head score matmuls on distinct 32-row PE bands;
                # head j output at bank-aligned col slot
                slot = 512 if len(hs) == 2 else n_q
                for j, h in enumerate(hs):
                    nc.tensor.matmul(
                        sc[:, j * slot : j * slot + n_q],
                        kts[32 * h : 32 * h + HEAD_DIM, q0 : q0 + 128],
                        qts[32 * h : 32 * h + HEAD_DIM, q0:SEQ],
                        start=True,
                        stop=True,
                        tile_position=(32 * h, 0),
                    )
                exps = expp.tile([128, 1024], F16, tag="exps")
                exp_tiles[(c, wi)] = exps
                nh = len(hs)
                s3 = sc[:, 0 : nh * slot].rearrange("p (m n) -> p m n", m=nh)[
                    :, :, 0:n_q
                ]
                e3 = exps[:, 0 : nh * n_q].rearrange("p (m n) -> p m n", m=nh)
                nc.scalar.activation(
                    e3,
                    s3,
                    mybir.ActivationFunctionType.Exp,
                    scale=1.0 / np.sqrt(HEAD_DIM),
                    bias=expbias[:, :],
                )
                # causal mask on the diag 128-block of each head
                diag = e3[:, :, 0:128]
                m_ap = mask[:, :]
                m_bc = bass.AP(
                    tensor=m_ap.tensor,
                    offset=m_ap.offset,
                    ap=[m_ap.ap[0], [0, nh], m_ap.ap[1]],
                )
                nc.vector.tensor_mul(diag, diag, m_bc)
                pend_att.append((c, wi))
                if len(pend_att) > 2:
                    emit_attended(*pend_att.pop(0))

            def emit_tail(c):
                """recip -> broadcast dma -> normalize-multiply -> yT -> store"""
                ap = att_ps[c % 2]
                # strided-partition AP over the 4 denominator rows 32h+16
                base = ap[HEAD_DIM : HEAD_DIM + 1, :]
                den = bass.AP(
                    tensor=base.tensor,
                    offset=base.offset,
                    ap=[[32 * base.ap[0][0], 4], base.ap[1]],
                )
                recs = recp.tile([4, SEQ], F32, tag="recs")
                nc.vector.reciprocal_approx_fast(out=recs[:, :], in_=den)
                rb = recp.tile([128, SEQ], F32, tag="rb")
                r_ap = recs[:, :]
                r_bc = bass.AP(
                    tensor=r_ap.tensor,
                    offset=r_ap.offset,
                    ap=[r_ap.ap[0], [0, 32], r_ap.ap[1]],
                )
                nc.sync.dma_start(out=rb[:, :], in_=r_bc)
                attn = attnp.tile([128, SEQ], F16, tag="attn")
                nc.gpsimd.tensor_mul(attn[:, :], ap[:, :], rb[:, :])
                return attn

            def emit_out(c, attn):
                nc.tensor.matmul(
                    yt_ps[:, :], wot[:, :], attn[:, :], start=True, stop=True
                )
                yts = attnp.tile([D, SEQ], F32, tag="yts")
                nc.gpsimd.tensor_copy(yts[:, :], yt_ps[:, :])
                nc.sync.dma_start(out=y_d[c % n_chunks], in_=yts[:, :])

            def emit_pipeline():
                n_virt = n_chunks * n_repeat
                load_xt(0)
                load_xt(1)
                attns = {}
                # bootstrap head(0)
                q0_ = emit_head_q(0)
                k0_ = emit_head_k(0)
                v0_ = emit_head_v(0)
                heads[0] = (q0_, k0_, v0_)
                for c in range(n_virt + 2):
                    # flush attended leftovers of c-1, then its tail
                    if 0 <= c - 1 < n_virt:
                        while pend_att and pend_att[0][0] == c - 1:
                            emit_attended(*pend_att.pop(0))
                        attns[c - 1] = emit_tail(c - 1)
                    if c + 2 < n_virt:
                        load_xt(c + 2)
                    if c < n_virt:
                        hc = c + 1
                        hq = hk = hv = None
                        for wi in range(len(WAVES)):
                            if hc < n_virt:
                                if wi == 1:
                                    hq = emit_head_q(hc)
                                elif wi == 3:
                                    hk = emit_head_k(hc)
                                elif wi == 5:
                                    hv = emit_head_v(hc)
                            emit_wave(c, wi)
                        if hc < n_virt:
                            heads[hc] = (hq, hk, hv)
                        heads.pop(c - 1, None)
                    # output of chunk c-1 at end of c (attn ready mid-c)
                    if 0 <= c - 1 < n_virt:
                        emit_out(c - 1, attns.pop(c - 1))

            if hw_loop > 1:
                with tc.For_i(0, hw_loop, 1):
                    emit_pipeline()
            else:
                emit_pipeline()
    nc.compile()
    return nc


def _prep_weights(Wq, Wk, Wv, Wo):
    # wqt[i, 32h+d] = Wq[16h+d, i] (zero-padded rows 16..31 of each band)
    def pad_wt(W):
        wt = np.zeros((D, 128), dtype=np.float16)
        wt.reshape(D, 4, 32)[:, :, :HEAD_DIM] = (
            np.asarray(W, dtype=np.float32).T.reshape(D, 4, HEAD_DIM)
        )
        return wt

    wqt = pad_wt(Wq)
    wkt = pad_wt(Wk)
    wvt = np.ascontiguousarray(np.asarray(Wv, dtype=np.float32).T).astype(np.float16)
    # wot[32h+d, o] = Wo[o, 16h+d]; helper rows (d>=16) zero
    wot = np.zeros((128, D), dtype=np.float16)
    wot.reshape(4, 32, D)[:, :HEAD_DIM, :] = (
        np.asarray(Wo, dtype=np.float32).T.reshape(4, HEAD_DIM, D)
    )
    mask = np.triu(np.ones((128, 128), dtype=np.float16))
    return wqt, wkt, wvt, wot, mask


def _prep_x(x_core):
    """[C, S, D] fp32 -> [C, D, S] fp16 contiguous."""
    return np.ascontiguousarray(x_core.transpose(0, 2, 1)).astype(np.float16)


_NC_CACHE = {}


def _get_nc(n_chunks, n_repeat=1, hw_loop=1):
    key = (n_chunks, n_repeat, hw_loop)
    if key not in _NC_CACHE:
        _NC_CACHE[key] = build_nc(n_chunks, n_repeat, hw_loop)
    return _NC_CACHE[key]


def run(x, Wq, Wk, Wv, Wo, trace=False, n_repeat=1):
    x = np.asarray(x, dtype=np.float32)
    B, C, S, d = x.shape
    assert (B, S, d) == (N_CORES, SEQ, D), f"unexpected shape {x.shape}"
    wqt, wkt, wvt, wot, mask = _prep_weights(Wq, Wk, Wv, Wo)
    nc = _get_nc(C, n_repeat)
    in_maps = []
    for b in range(N_CORES):
        in_maps.append(
            {"xt": _prep_x(x[b]), "wqt": wqt, "wkt": wkt, "wvt": wvt,
             "wot": wot, "mask": mask}
        )
    res = run_bass_kernel_spmd(nc, in_maps, list(range(N_CORES)), trace=trace)
    # yT [C, 64, 512] -> y [C, 512, 64]
    y = np.stack(
        [res.results[b]["y"].transpose(0, 2, 1) for b in range(N_CORES)], axis=0
    )
    return y, res


def kernel(x, Wq, Wk, Wv, Wo):
    y, _ = run(x, Wq, Wk, Wv, Wo, trace=False)
    return y
